# revision 4
# baseline (speedup 1.0000x reference)
"""GQA attention kernel v2 for Trainium2, 8-core tensor-parallel (by heads).

Same sharding as v1 (core c owns q heads [4c,4c+4) and kv head c), but:
  - all big tensors (x, wq/wk/wv, wo, out) in bf16 -> half the HBM traffic
  - QKV projections computed directly in transposed [feat, token] layout
    (weights stationary, x moving) so q/k/v need no per-head PE transposes
  - RoPE applied in transposed layout via a pair-swap permutation matmul
    plus elementwise cos/sin multiplies
  - output partials stored bf16; host sums the 8 partials and adds bo
"""
import sys
sys.path.insert(0, "/opt/trn_rl_repo")

import numpy as np

B, S, D = 4, 128, 4096
H, KV, HD = 32, 8, 128
NCORES = 8
HQ = H // NCORES          # 4 q heads per core
T = B * S                 # 512 tokens
FQ = HQ * HD              # 512 q features per core
NK = D // 128             # 32 contraction chunks
SCALE = 1.0 / float(np.sqrt(HD))

_CACHE = {}


def _build():
    import concourse.bass as bass
    import concourse.tile as tile
    from concourse import bacc, mybir

    F32 = mybir.dt.float32
    BF16 = mybir.dt.bfloat16
    AF = mybir.ActivationFunctionType

    nc = bacc.Bacc("TRN2", target_bir_lowering=False, debug=False,
                   enable_asserts=False, num_devices=NCORES)

    xs_d = nc.dram_tensor("xs", [128, NK * T], BF16, kind="ExternalInput").ap()
    wqkv_d = nc.dram_tensor("wqkv", [128, NK * 768], BF16, kind="ExternalInput").ap()
    wo_d = nc.dram_tensor("wo", [FQ, D], BF16, kind="ExternalInput").ap()
    cosT_d = nc.dram_tensor("cosT", [128, T], BF16, kind="ExternalInput").ap()
    sinT_d = nc.dram_tensor("sinT", [128, T], BF16, kind="ExternalInput").ap()
    mkT_d = nc.dram_tensor("mkT", [128, HQ * S], BF16, kind="ExternalInput").ap()
    ones_d = nc.dram_tensor("ones", [128, S], BF16, kind="ExternalInput").ap()
    identT_d = nc.dram_tensor("identT", [128, S], BF16, kind="ExternalInput").ap()
    pswap_d = nc.dram_tensor("pswap", [128, S], BF16, kind="ExternalInput").ap()
    bqT_d = nc.dram_tensor("bqT", [128, HQ], F32, kind="ExternalInput").ap()
    bkvT_d = nc.dram_tensor("bkvT", [128, 2], F32, kind="ExternalInput").ap()
    out_d = nc.dram_tensor("out", [T, D], BF16, kind="ExternalOutput").ap()

    # k-chunk DMA group sizes: small first groups so PE starts early
    GROUPS = [1, 1, 2, 4, 6, 6, 6, 6]
    assert sum(GROUPS) == NK

    with tile.TileContext(nc) as tc:
        with tc.tile_pool(name="consts", bufs=1) as cp:
            xs = cp.tile([128, NK * T], BF16)
            wq_s = cp.tile([128, NK * 768], BF16)
            wo_s = [cp.tile([128, D], BF16, name=f"wo{h}") for h in range(HQ)]
            cosT = cp.tile([128, T], BF16)
            sinT = cp.tile([128, T], BF16)
            mkT = cp.tile([128, HQ * S], BF16)
            ones = cp.tile([128, S], BF16)
            identT = cp.tile([128, S], BF16)
            pswap = cp.tile([128, S], BF16)
            bqT = cp.tile([128, HQ], F32)
            bkvT = cp.tile([128, 2], F32)

            # weights stream on the gpsimd (SWDGE) queue, x on the sync
            # (HWDGE-SP) queue; consts on scalar (HWDGE-ACT). Output stores
            # reuse SP later.
            k0 = 0
            for g, kg in enumerate(GROUPS):
                nc.gpsimd.dma_start(wq_s[:, k0 * 768:(k0 + kg) * 768],
                                    wqkv_d[:, k0 * 768:(k0 + kg) * 768])
                nc.sync.dma_start(xs[:, k0 * T:(k0 + kg) * T],
                                  xs_d[:, k0 * T:(k0 + kg) * T])
                k0 += kg
            for h in range(HQ):
                nc.gpsimd.dma_start(wo_s[h], wo_d[h * 128:(h + 1) * 128, :])
            nc.scalar.dma_start(cosT, cosT_d)
            nc.scalar.dma_start(sinT, sinT_d)
            nc.scalar.dma_start(mkT, mkT_d)
            nc.scalar.dma_start(ones, ones_d)
            nc.scalar.dma_start(identT, identT_d)
            nc.scalar.dma_start(pswap, pswap_d)
            nc.scalar.dma_start(bqT, bqT_d)
            nc.scalar.dma_start(bkvT, bkvT_d)

            with tc.tile_pool(name="sb", bufs=1) as sp, \
                 tc.tile_pool(name="tmp", bufs=2) as tp, \
                 tc.tile_pool(name="attn", bufs=2) as ap_, \
                 tc.tile_pool(name="aop", bufs=4) as aop, \
                 tc.tile_pool(name="outp", bufs=4) as op:

                # ---------- Phase A: QKV projections (transposed out) ------
                qTs = [sp.tile([128, T], BF16, name=f"qT{h}") for h in range(HQ)]
                kT_pre = sp.tile([128, T], BF16, name="kT_pre")
                vT = sp.tile([128, T], BF16, name="vT")
                with tc.tile_pool(name="psA", bufs=1, space="PSUM") as psA:
                    pq = [psA.tile([128, T], F32, tag=f"pq{h}", name=f"pq{h}")
                          for h in range(HQ)]
                    pk = psA.tile([128, T], F32, tag="pk", name="pk")
                    pv = psA.tile([128, T], F32, tag="pv", name="pv")
                    # chunks 0..15: per-chunk order (matches streaming DMA
                    # arrival); chunks 16..31: per-OUTPUT passes (weights all
                    # resident by then) so k/v and each q head finish early
                    # and their drain+RoPE hides under the next pass's GEMMs.
                    KSPLIT = NK // 2
                    outs_a = ([(pk, 512, 640), (pv, 640, 768)] +
                              [(pq[h], h * 128, (h + 1) * 128) for h in range(HQ)])
                    for k in range(KSPLIT):
                        rhs = xs[:, k * T:(k + 1) * T]
                        base = k * 768
                        st = (k == 0)
                        for dst_, lo, hi in outs_a:
                            nc.tensor.matmul(dst_, wq_s[:, base + lo:base + hi],
                                             rhs, start=st, stop=False)
                    # ------ Phase A2 + B interleaved ------
                    # per-output passes over chunks 16..31; each output's
                    # drain + RoPE (or v transposes) is emitted right after
                    # its pass so it hides under the next pass's GEMMs. The
                    # swp-tag banks plus the six accumulation banks fill PSUM
                    # exactly (psA pool stays open; no pool-close barrier).
                    qrT = sp.tile([128, HQ * T], BF16, name="qrT")
                    krT = sp.tile([128, T], BF16, name="krT")
                    v_m = [None] * B

                    def a2_pass(dst_, lo, hi):
                        for k in range(KSPLIT, NK):
                            rhs = xs[:, k * T:(k + 1) * T]
                            base = k * 768
                            nc.tensor.matmul(dst_, wq_s[:, base + lo:base + hi],
                                             rhs, start=False,
                                             stop=(k == NK - 1))

                    def rope(h, last=False):
                        src = (qTs[h] if h < HQ else kT_pre)
                        dst = (qrT[:, h * T:(h + 1) * T] if h < HQ else krT)
                        pswp = psA.tile([128, T], F32, tag="swp", bufs=2,
                                        name=f"pswp{h}")
                        nc.tensor.matmul(pswp, pswap, src, start=True, stop=True)
                        t1 = tp.tile([128, T], BF16, tag="t1", name=f"t1_{h}")
                        nc.gpsimd.tensor_mul(t1, src, cosT)
                        t2 = tp.tile([128, T], BF16, tag="t2", name=f"t2_{h}")
                        if last:
                            # final head is latency-critical: skip the ACT
                            # drain hop, multiply straight out of PSUM on DVE
                            nc.vector.tensor_mul(t2, pswp, sinT)
                        else:
                            # steady state: ACT drains PSUM, Pool does the cos
                            # mul, DVE (4x bf16) the sin mul + add
                            t2s = tp.tile([128, T], BF16, tag="t2s",
                                          name=f"t2s{h}")
                            nc.scalar.copy(t2s, pswp)
                            nc.vector.tensor_mul(t2, t2s, sinT)
                        nc.vector.tensor_add(dst, t1, t2)

                    # pass k -> drain + rope(k)
                    a2_pass(pk, 512, 640)
                    nc.scalar.activation(kT_pre, pk, AF.Identity, bias=bkvT[:, 0:1])
                    rope(HQ)
                    # pass v -> drain + transposes
                    a2_pass(pv, 640, 768)
                    nc.vector.tensor_scalar_add(vT, pv, bkvT[:, 1:2])
                    pvm = psA.tile([128, T], BF16, tag="swp", bufs=2,
                                   name="pvm")
                    for m in range(B):
                        nc.tensor.transpose(pvm[:, m * S:(m + 1) * S],
                                            vT[:, m * S:(m + 1) * S], identT)
                    v_all = sp.tile([128, T], BF16, name="v_all")
                    nc.vector.tensor_copy(v_all, pvm)
                    for m in range(B):
                        v_m[m] = v_all[:, m * S:(m + 1) * S]
                    # q-head passes -> drain + rope each; the last head's
                    # bias goes to ACT (DVE is its critical path)
                    for h in range(HQ):
                        a2_pass(pq[h], h * 128, (h + 1) * 128)
                        if h % 2 == 1 and h != HQ - 1:
                            nc.vector.tensor_scalar_add(qTs[h], pq[h],
                                                        bqT[:, h:h + 1])
                        else:
                            nc.scalar.activation(qTs[h], pq[h],
                                                 AF.Identity, bias=bqT[:, h:h + 1])
                        rope(h, last=(h == HQ - 1))

                if True:
                    # ---------- Phase C: attention per batch ----------
                    # po_u = V^T @ (exp(scores) * mask) runs in parallel with
                    # the denominator matmul; normalization folds into the
                    # PSUM->SBUF copy (rec rows are all equal).
                    qv = qrT.rearrange("p (h t) -> p h t", h=HQ)
                    aoT = [None] * B
                    for m in range(B):
                        psc = psA.tile([128, HQ * S], F32, tag=f"pq{m}",
                                       name=f"psc{m}")
                        nc.tensor.matmul(psc, krT[:, m * S:(m + 1) * S],
                                         qv[:, :, m * S:(m + 1) * S],
                                         start=True, stop=True)
                        eu = ap_.tile([128, HQ * S], BF16, tag="eu", name=f"eu{m}")
                        nc.scalar.activation(eu, psc, AF.Exp, scale=SCALE)
                        au = ap_.tile([128, HQ * S], BF16, tag="au", name=f"au{m}")
                        nc.gpsimd.tensor_mul(au, eu, mkT)
                        pden = psA.tile([128, HQ * S], F32, tag="pk",
                                        name=f"pden{m}")
                        nc.tensor.matmul(pden, ones, au, start=True, stop=True)
                        po = psA.tile([128, HQ * S], F32, tag="pv",
                                      name=f"po{m}")
                        nc.tensor.matmul(po, v_m[m], au, start=True, stop=True)
                        rec = ap_.tile([128, HQ * S], F32, tag="rec", name=f"rec{m}")
                        nc.vector.reciprocal(rec, pden)
                        aoT[m] = aop.tile([128, HQ * S], BF16, tag="aoT",
                                          name=f"aoT{m}")
                        nc.vector.tensor_mul(aoT[m], po, rec)

                    # ---------- Phase D: output projection ----------
                    NT = D // 512
                    for m in range(B):
                        outm = op.tile([128, D], BF16, tag="outm", name=f"outm{m}")
                        for n in range(NT):
                            pso = psA.tile([128, 512], F32, tag="swp", bufs=2,
                                           name=f"pso{m}_{n}")
                            for h in range(HQ):
                                nc.tensor.matmul(
                                    pso, aoT[m][:, h * 128:(h + 1) * 128],
                                    wo_s[h][:, n * 512:(n + 1) * 512],
                                    start=(h == 0), stop=(h == HQ - 1))
                            if (m * NT + n) % 2 == 0:
                                nc.vector.tensor_copy(
                                    outm[:, n * 512:(n + 1) * 512], pso)
                            else:
                                nc.scalar.copy(
                                    outm[:, n * 512:(n + 1) * 512], pso)
                            if m == B - 1:
                                # last batch: store per n-tile to shorten the
                                # final copy->DMA->drain chain
                                nc.sync.dma_start(
                                    out_d[m * S:(m + 1) * S,
                                          n * 512:(n + 1) * 512],
                                    outm[:, n * 512:(n + 1) * 512])
                            elif n % 2 == 1:
                                # store finished 1024-col quarter immediately
                                qlo = (n - 1) * 512
                                nc.sync.dma_start(
                                    out_d[m * S:(m + 1) * S, qlo:qlo + 1024],
                                    outm[:, qlo:qlo + 1024])

    nc.compile()
    return nc


def _prep_inputs(x, freqs_cos, freqs_sin, wq, bq, wk, bk, wv, bv, wo):
    from ml_dtypes import bfloat16 as bf16

    xT = x.reshape(T, D).T.astype(np.float32)                  # (D, T)
    xs = np.ascontiguousarray(
        xT.reshape(NK, 128, T).transpose(1, 0, 2).reshape(128, NK * T)
    ).astype(bf16)

    cos_d = np.repeat(freqs_cos.astype(np.float32), 2, axis=1)  # (S, 128)
    sin_d = np.repeat(freqs_sin.astype(np.float32), 2, axis=1)
    sign = np.tile(np.array([-1.0, 1.0], np.float32), HD // 2)
    cosT = np.ascontiguousarray(np.tile(cos_d.T, (1, B))).astype(bf16)  # (128, T)
    sinT = np.ascontiguousarray(
        np.tile((sin_d * sign[None, :]).T, (1, B))).astype(bf16)
    mkT = np.ascontiguousarray(
        np.tile(np.triu(np.ones((S, S), np.float32)), (1, HQ))).astype(bf16)
    ones = np.ones((S, S), np.float32).astype(bf16)
    identT = np.eye(S, dtype=np.float32).astype(bf16)
    pswap = np.kron(np.eye(HD // 2, dtype=np.float32),
                    np.array([[0, 1], [1, 0]], np.float32)).astype(bf16)

    maps = []
    for c in range(NCORES):
        qs = slice(c * FQ, (c + 1) * FQ)
        ks = slice(c * HD, (c + 1) * HD)
        wqkv = np.concatenate(
            [wq[:, qs], wk[:, ks], wv[:, ks]], axis=1).astype(np.float32)  # (D, 768)
        wqkv_t = np.ascontiguousarray(
            wqkv.reshape(NK, 128, 768).transpose(1, 0, 2).reshape(128, NK * 768)
        ).astype(bf16)
        bqT = np.ascontiguousarray(
            bq[qs].astype(np.float32).reshape(HQ, HD).T)       # (128, HQ)
        bkvT = np.ascontiguousarray(
            np.stack([bk[ks], bv[ks]], axis=1).astype(np.float32))  # (128, 2)
        maps.append({
            "xs": xs,
            "wqkv": wqkv_t,
            "wo": np.ascontiguousarray(wo[qs, :].astype(np.float32)).astype(bf16),
            "cosT": cosT, "sinT": sinT, "mkT": mkT, "ones": ones,
            "identT": identT, "pswap": pswap, "bqT": bqT, "bkvT": bkvT,
        })
    return maps


def kernel(x, start_pos, freqs_cos, freqs_sin, mask, cache_k, cache_v,
           wq, bq, wk, bk, wv, bv, wo, bo, _want_trace=False):
    from concourse.bass_utils import run_bass_kernel_spmd

    assert int(start_pos) == 0
    if "nc" not in _CACHE:
        _CACHE["nc"] = _build()
    nc = _CACHE["nc"]
    in_maps = _prep_inputs(np.asarray(x), np.asarray(freqs_cos),
                           np.asarray(freqs_sin), np.asarray(wq),
                           np.asarray(bq), np.asarray(wk), np.asarray(bk),
                           np.asarray(wv), np.asarray(bv), np.asarray(wo))
    res = run_bass_kernel_spmd(nc, in_maps, core_ids=list(range(NCORES)),
                               trace=_want_trace)
    acc = np.zeros((T, D), np.float64)
    for r in res.results:
        acc += r["out"].astype(np.float64)
    out = (acc + np.asarray(bo).astype(np.float64)).astype(np.float32)
    if _want_trace:
        _CACHE["last_exec_time_ns"] = res.exec_time_ns
        _CACHE["last_trace"] = res.instructions_and_trace
    return out.reshape(B, S, D)
